# revision 1
# baseline (speedup 1.0000x reference)
"""Trainium2 Bass kernel for a single causal attention head.

Problem: x:[8,2048,1024] f32, Wq/Wk/Wv:[64,1024], causal mask.
  Q = x@Wq.T; K = x@Wk.T; V = x@Wv.T
  out = softmax(mask(Q@K.T/sqrt(64))) @ V          -> [8, 2048, 64] f32

Sharding: data-parallel over batch. B == n_cores == 8, so each NeuronCore
computes one full batch element; no collectives.

Per-core algorithm (fp16 matmul inputs, fp32 PSUM accumulation):
  - x is passed per-core as fp16 [S,E]; the device xbar-DMA-transposes it
    into xT [E,S] chunks (contraction dim e must sit on SBUF partitions).
  - Fused projection [Wq;Wv]: psum rows 0:64 = QT, 64:128 = VT.
    Separate Wk projection -> KT [64, S].  All as lhsT.T @ xT matmuls, N=512.
  - VT tiles are PE-transposed back to V [k,128] and augmented with a ones
    column -> V_aug [128, 65].
  - scoresT[k,q] = KT_chunk.T @ QT (K=64 contraction), causally skipped at
    (128k x 512q) block granularity.  exp(0.125*s) on ScalarE straight out
    of PSUM (scores are O(1), no max-subtraction needed); staircase masks
    multiply the 4 diagonal-block shapes (exact zeros, matching -inf mask).
  - out_augT[65,q] = V_aug.T @ expT accumulated over k tiles: rows 0:64 are
    the unnormalized outT, row 64 is the softmax denominator Z (from the
    ones column).
  - 1/Z broadcast to 64 partitions via a K=1 matmul, multiply, DMA outT
    [64, S] f32; host transposes back.
"""

import numpy as np

B, S, E, D = 8, 2048, 1024, 64
NCORES = 8
EC = E // 128   # 8 e-chunks
ST = S // 128   # 16 s(=k)-tiles
QB = S // 512   # 4 q-blocks

_cache = {}


def _patch_tile_drain():
    """The pinned walrus rejects >~2 sem waits on one Drain; Tile's tail
    drain waits on every live semaphore.  Split the excess onto standalone
    wait_ge instructions (same semantics: all waits complete before the
    all-engine barrier resets semaphores)."""
    import concourse.mybir as mybir
    import concourse.tile as ctile
    from concourse.vector_clock import ScopedClock

    if getattr(ctile.TileContext, "_drain_patch", False):
        return

    def _drain_and_barrier(self, tick_clock, wait_clock):
        nc = self.nc
        drain_inst = nc.sync.drain()
        wait_clock.add_sem_waits(
            drain_inst.ins, ScopedClock({None: tick_clock.global_clock})
        )
        si = drain_inst.ins.sync_info
        if si is not None and si.on_wait and len(si.on_wait) > 1:
            waits = list(si.on_wait)
            drain_inst.ins.sync_info = mybir.SyncInfo(
                on_wait=[waits[0]], on_update=list(si.on_update)
            )
            handles = {h.num: h for h in self.sems.allocated().values()}
            for w in waits[1:]:
                assert w.wait_mode == "sem-ge-imm", w
                nc.sync.wait_ge(handles[w.id], w.wait_value)
        nc.all_engine_barrier()
        popped = nc._tile_sem_poison_stack.pop()
        assert popped is self._sem_poison
        nc.clear_and_free_semaphores(list(self.sems.allocated().values()))
        nc.all_engine_barrier()

    ctile.TileContext._drain_and_barrier = _drain_and_barrier
    ctile.TileContext._drain_patch = True


def _split_sync_waits(nc, maxw=1):
    """The pinned walrus rejects instructions carrying more than ~2 sem
    waits.  Hoist all but `maxw` waits of every instruction onto dedicated
    NoOps just before it in the same engine stream (engine streams are
    in-order, so semantics are identical)."""
    import concourse.mybir as mybir

    n_new = 0
    for f in nc.m.functions:
        for b in f.blocks:
            out = []
            changed = False
            for inst in b.instructions:
                si = getattr(inst, "sync_info", None)
                if si is not None and si.on_wait and len(si.on_wait) > maxw:
                    waits = list(si.on_wait)
                    extras, keep = waits[:-maxw], waits[-maxw:]
                    for k, w in enumerate(extras):
                        nop = mybir.InstNoOp(
                            name=f"{inst.name}-hw{k}", ins=[], outs=[],
                            sync_info=mybir.SyncInfo(on_wait=[w], on_update=[]),
                        )
                        nop.engine = inst.engine
                        nc.register_instruction(nop)
                        out.append(nop)
                        n_new += 1
                    inst.sync_info = mybir.SyncInfo(
                        on_wait=keep, on_update=list(si.on_update)
                    )
                    changed = True
                out.append(inst)
            if changed:
                b.instructions = out
    return n_new


def _build_nc():
    import concourse.bass as bass
    import concourse.mybir as mybir
    from concourse import tile
    from concourse.masks import make_identity

    _patch_tile_drain()

    fp16 = mybir.dt.float16
    f32 = mybir.dt.float32
    EXP = mybir.ActivationFunctionType.Exp

    nc = bass.Bass("TRN2", target_bir_lowering=False)
    # all inputs pre-laid-out on host: xT = x.T, wqvT = [Wq;Wv].T, wkT = Wk.T
    xT_d = nc.dram_tensor("xT", [E, S], fp16, kind="ExternalInput")
    wqvT_d = nc.dram_tensor("wqvT", [E, 128], fp16, kind="ExternalInput")
    wkT_d = nc.dram_tensor("wkT", [E, D], fp16, kind="ExternalInput")
    mask_d = nc.dram_tensor("maskt", [128, 4, 512], fp16, kind="ExternalInput")
    out_d = nc.dram_tensor("out", [S, D], f32, kind="ExternalOutput")

    with tile.TileContext(nc) as tc:
        with (
            tc.tile_pool(name="singles", bufs=1) as singles,
            tc.tile_pool(name="sb", bufs=3) as sb,
            tc.tile_pool(name="expp", bufs=6) as expp,
            tc.tile_pool(name="psA", bufs=2, space="PSUM") as psA,
            tc.tile_pool(name="psS", bufs=2, space="PSUM") as psS,
            tc.tile_pool(name="psO", bufs=2, space="PSUM") as psO,
            tc.tile_pool(name="psT", bufs=2, space="PSUM") as psT,
        ):
            # ---- constants / inputs ----
            ident = singles.tile([128, 128], fp16)
            make_identity(nc, ident[:])
            maskt = singles.tile([128, 4, 512], fp16)
            nc.sync.dma_start(maskt[:], mask_d[:])

            xT = singles.tile([128, EC, S], fp16)          # [e-chunk part, ec, s]
            for ec in range(EC):
                nc.sync.dma_start(xT[:, ec, :], xT_d[ec * 128:(ec + 1) * 128, :])
            wqvT = singles.tile([128, EC, 128], fp16)
            wkT = singles.tile([128, EC, D], fp16)
            for ec in range(EC):
                nc.sync.dma_start(wqvT[:, ec, :], wqvT_d[ec * 128:(ec + 1) * 128, :])
                nc.sync.dma_start(wkT[:, ec, :], wkT_d[ec * 128:(ec + 1) * 128, :])

            # ---- projections: qv_sb rows 0:64 = QT, 64:128 = VT;  kt = KT ----
            qv_sb = singles.tile([128, S], fp16)
            kt = singles.tile([64, S], fp16)
            for jb in range(QB):
                qs = slice(jb * 512, (jb + 1) * 512)
                ps_qv = psA.tile([128, 512], f32, tag="proj", name=f"ps_qv{jb}")
                for ec in range(EC):
                    nc.tensor.matmul(ps_qv[:], wqvT[:, ec, :], xT[:, ec, qs],
                                     start=(ec == 0), stop=(ec == EC - 1))
                nc.scalar.copy(qv_sb[:, qs], ps_qv[:])
                ps_k = psA.tile([64, 512], f32, tag="proj", name=f"ps_k{jb}")
                for ec in range(EC):
                    nc.tensor.matmul(ps_k[:], wkT[:, ec, :], xT[:, ec, qs],
                                     start=(ec == 0), stop=(ec == EC - 1))
                nc.scalar.copy(kt[:, qs], ps_k[:])

            # ---- V_aug tiles: [k 128, 65] with ones in col 64 ----
            vaug = singles.tile([128, ST, 65], fp16)
            for si in range(ST):
                ps_t = psT.tile([128, 64], fp16, tag="tr", name=f"ps_vt{si}")
                nc.tensor.transpose(ps_t[:], qv_sb[64:128, si * 128:(si + 1) * 128],
                                    ident[64:128, 64:128])
                nc.vector.tensor_copy(vaug[:, si, 0:64], ps_t[:])
                nc.vector.memset(vaug[:, si, 64:65], 1.0)

            # ---- attention ----
            for jb in range(QB):
                qs = slice(jb * 512, (jb + 1) * 512)
                ps_o = psO.tile([65, 512], f32, tag="o", name=f"ps_o{jb}")
                nki = 4 * jb + 4
                for ki in range(nki):
                    ps_s = psS.tile([128, 512], f32, tag="s", name=f"ps_s{jb}_{ki}")
                    nc.tensor.matmul(ps_s[:], kt[:, ki * 128:(ki + 1) * 128],
                                     qv_sb[0:64, qs], start=True, stop=True)
                    ex = expp.tile([128, 512], fp16, tag="ex", name=f"ex{jb}_{ki}")
                    r = ki - 4 * jb
                    if r >= 0:  # diagonal block: exp then multiplicative mask
                        ex0 = expp.tile([128, 512], fp16, tag="ex0", name=f"ex0{jb}_{ki}")
                        nc.scalar.activation(ex0[:], ps_s[:], EXP, scale=0.125)
                        nc.vector.tensor_mul(ex[:], ex0[:], maskt[:, r, :])
                    else:
                        nc.scalar.activation(ex[:], ps_s[:], EXP, scale=0.125)
                    nc.tensor.matmul(ps_o[:], vaug[:, ki, :], ex[:],
                                     start=(ki == 0), stop=(ki == nki - 1))
                # normalize: transpose [65,512] -> 4x [128,65] (row 64 -> col 64
                # = Z per query), then per-partition 1/Z scale, DMA out
                outT_sb = sb.tile([65, 512], fp16, tag="outTs", name=f"outTs{jb}")
                nc.scalar.copy(outT_sb[:], ps_o[:])
                for sub in range(4):
                    si = jb * 4 + sub
                    ps_f = psT.tile([128, 65], fp16, tag="tr", name=f"ps_f{si}")
                    nc.tensor.transpose(ps_f[:], outT_sb[:, sub * 128:(sub + 1) * 128],
                                        ident[0:65, 0:65])
                    recip = sb.tile([128, 1], f32, tag="recip", name=f"recip{si}")
                    nc.vector.reciprocal(recip[:], ps_f[:, 64:65])
                    o_sb = sb.tile([128, 64], f32, tag="of", name=f"of{si}")
                    nc.vector.tensor_scalar_mul(o_sb[:], ps_f[:, 0:64], recip[:])
                    nc.sync.dma_start(out_d[si * 128:(si + 1) * 128, :], o_sb[:])
    _split_sync_waits(nc)
    nc.finalize()
    return nc


def _host_mask():
    kp = np.arange(128)[:, None, None]
    r = np.arange(4)[None, :, None]
    f = np.arange(512)[None, None, :]
    return (kp + 128 * r <= f).astype(np.float16)


def kernel(x, Wq, Wk, Wv, attention_mask=None, **_unused):
    from concourse.bass_utils import run_bass_kernel_spmd

    if "nc" not in _cache:
        _cache["nc"] = _build_nc()
    nc = _cache["nc"]

    wqvT = np.ascontiguousarray(
        np.concatenate([np.asarray(Wq), np.asarray(Wv)], 0).T
    ).astype(np.float16)
    wkT = np.ascontiguousarray(np.asarray(Wk).T).astype(np.float16)
    maskt = _host_mask()
    x = np.asarray(x)
    in_maps = [
        {
            "xT": np.ascontiguousarray(x[b].T).astype(np.float16),
            "wqvT": wqvT,
            "wkT": wkT,
            "maskt": maskt,
        }
        for b in range(B)
    ]
    import os

    tmpdir = None
    if os.environ.get("BASS_TRACE"):
        tmpdir = os.environ.get("BASS_TRACE_DIR", "/tmp/bass_trace")
        os.makedirs(tmpdir, exist_ok=True)
    res = run_bass_kernel_spmd(nc, in_maps, core_ids=list(range(NCORES)), tmpdir=tmpdir)
    out = np.stack([res.results[b]["out"] for b in range(B)], 0)
    _cache["last_exec_time_ns"] = res.exec_time_ns
    _cache["trace_dir"] = tmpdir
    return out.astype(np.float32)

